# revision 28
# baseline (speedup 1.0000x reference)
"""Linear attention (B=4, S=4096, D=1024, H=16) on 8 TRN2 NeuronCores.

Sharding: core = (batch, head-half): each core handles one batch's 8 heads.
 - x is host-transposed to xT [D, S] per batch so both operand orientations
   of every matmul come out of the tensor engine with no on-device transpose.
 - Wqkv column-sharded per head-half; Wo row-sharded; host sums the two
   partial y's per batch (row-parallel unshard).

v2: K and Q projections run in fp8e4 DoubleRow (2 contraction rows per PE
cell, ~1.8x per-matmul throughput). Errors in K/Q largely cancel through
the attention normalizer (measured end-to-end ~1.25e-2 vs 2e-2 budget);
V/out-proj paths stay bf16 (their fp8 error flows straight to the output).
Host ships x8 = e4m3(xT*16) and wqk8 = e4m3([Wq|Wk]*512); the 1/8192
descale folds into the ACT activation scale of the elu evaluation, and the
K-path relu becomes a DVE scalar_tensor_tensor multiply with a constant
tile, so ACT/DVE load is unchanged vs the bf16 kernel.

Two-phase dataflow (V/out matmuls bf16, fp32 PSUM accumulate):

phase 1 (per 512-token block): K projection fp8-DR (4 k-pair matmuls per
  128-token subtile) -> elu+1(K); V projection bf16 (8 k matmuls) ->
  [KV | K_sum^T] PSUM accumulation per head-pair (vst carries a ones
  column so one matmul does both). bf16 x is streamed per block (4-deep
  pool) since only the V projection reads it; x8 stays fully resident.
  Block 0 runs the K projection k-pair-outer (4 simultaneous PSUM chains)
  so compute starts as soon as the first (x8, wk8) DMA chunk lands.
  Block 0's Q projection runs at the end of phase 1 to bridge the
  phase transition with PE work that has no KV dependency.

phase 2 (per block, software-pipelined across j):
  QT [512f, 512s] feature-major fp8-DR (lhsT=wq8 pair, rhs=x8 pair) ->
  elu+1 -> bf16
  psc[128,s] = blockdiag(KV_h0, KV_h1)^T @ QT_pair (bf16); norm via the
  replicated-ksum matmul; 1/x on DVE fast reciprocal; outT = outu * rcp
  y[s,:] = outT^T @ Wo per 128-token subtile, fp32, one 512KB DMA per
  subtile.
"""

import numpy as np

import concourse.bacc as bacc
import concourse.mybir as mybir
import concourse.tile as tile
from concourse.bass_utils import run_bass_kernel_spmd

F32 = mybir.dt.float32
BF16 = mybir.dt.bfloat16
F8 = mybir.dt.float8e4
ACT = mybir.ActivationFunctionType
DR = mybir.MatmulPerfMode.DoubleRow

P = 128
B, S, D = 4, 4096, 1024
H = 16
HD = 64

FSH = 512            # features per core for each of Q, K, V (8 heads)
KSUB = D // P        # 8 contraction subtiles
KPAIR = KSUB // 2    # 4 fp8 DoubleRow contraction pairs
SBLK = 512           # tokens per block
NBLK = S // SBLK     # 8 blocks
TSUB = SBLK // P     # 4 token subtiles per block
NPAIR = 4            # head pairs per core

SX = 16.0            # fp8 pre-scale on x
SW = 512.0           # fp8 pre-scale on Wq/Wk
INV = 1.0 / (SX * SW)

_NC_CACHE = None


def build():
    nc = bacc.Bacc(target_bir_lowering=False)
    xT = nc.dram_tensor("xT", [D, S], BF16, kind="ExternalInput")
    wqk8 = nc.dram_tensor("wqk8", [D, 2 * FSH], F8, kind="ExternalInput")
    wv = nc.dram_tensor("wv", [D, FSH], BF16, kind="ExternalInput")
    wo = nc.dram_tensor("wo", [FSH, D], BF16, kind="ExternalInput")
    y = nc.dram_tensor("y", [S, D], F32, kind="ExternalOutput")

    xT_r = xT.rearrange("(ko p) s -> p ko s", p=P)        # [128, 8, 4096]
    wqk8_r = wqk8.rearrange("(ko p) f -> p ko f", p=P)    # [128, 8, 1024]
    wv_r = wv.rearrange("(ko p) f -> p ko f", p=P)        # [128, 8, 512]
    wo_r = wo.rearrange("(fo p) n -> p fo n", p=P)        # [128, 4, 1024]
    y_rt = y.rearrange(
        "(j t p) (nh n) -> j t nh p n", t=TSUB, p=P, nh=2
    )  # [8,4,2,128,512]
    y_rb = y.rearrange("(j t p) d -> j t p d", t=TSUB, p=P)  # [8,4,128,1024]

    with tile.TileContext(nc) as tc:
        import contextlib

        with contextlib.ExitStack() as ctx:
            wpool = ctx.enter_context(tc.tile_pool(name="wpool", bufs=1))

            # persistent SBUF
            x8_sb = wpool.tile([P, KSUB, S], F8)            # all of x8, 32KB/p
            wqk8_sb = wpool.tile([P, KSUB, 2 * FSH], F8)    # [wq8|wk8]
            wv_sb = wpool.tile([P, KSUB, FSH], BF16)
            wo_sb = wpool.tile([P, FSH // P, D], BF16)
            cinv = wpool.tile([P, SBLK], F32)               # INV const tile
            # per-pair block-diagonal [[KV_h0, 0], [0, KV_h1]] (128x128)
            lhsT2_sb = [
                wpool.tile([P, P], BF16, name=f"l2{p}") for p in range(NPAIR)
            ]
            # per-pair [ksum_h0 x64 | ksum_h1 x64] replicated along free dim
            ksumrep_sb = [
                wpool.tile([P, P], BF16, name=f"kr{p}") for p in range(NPAIR)
            ]

            xbfpool = ctx.enter_context(tc.tile_pool(name="xbf", bufs=4))
            xbfs = []

            # x8 is derived on-chip (DVE cast of the streamed bf16 x, per
            # block) instead of shipped from HBM — saves 4MB of
            # startup-critical DMA. Block 0's x arrives per k-pair
            # interleaved with the wk8 pairs so the K projection starts on
            # the first chunks; everything else queues behind on the
            # in-order sync queue.
            xbf0 = xbfpool.tile([P, KSUB, SBLK], BF16, tag="xbf", name="xbf0")
            xbfs.append(xbf0)
            for i in range(KPAIR):
                nc.sync.dma_start(
                    out=xbf0[:, 2 * i : 2 * i + 2, :],
                    in_=xT_r[:, 2 * i : 2 * i + 2, 0:SBLK],
                )
                nc.sync.dma_start(
                    out=wqk8_sb[:, 2 * i : 2 * i + 2, FSH : 2 * FSH],
                    in_=wqk8_r[:, 2 * i : 2 * i + 2, FSH : 2 * FSH],
                )
                nc.sync.dma_start(
                    out=wv_sb[:, 2 * i : 2 * i + 2, :],
                    in_=wv_r[:, 2 * i : 2 * i + 2, :],
                )

            def xbf_dma(j):
                xb = xbfpool.tile([P, KSUB, SBLK], BF16, tag="xbf", name=f"xbf{j}")
                xbfs.append(xb)
                nc.sync.dma_start(
                    out=xb, in_=xT_r[:, :, j * SBLK : (j + 1) * SBLK]
                )

            # wq8/wo are not needed until the very end of phase 1, so they
            # queue behind the first two x prefetches
            xbf_dma(1)
            nc.sync.dma_start(
                out=wqk8_sb[:, :, 0:FSH], in_=wqk8_r[:, :, 0:FSH]
            )
            xbf_dma(2)
            nc.sync.dma_start(out=wo_sb, in_=wo_r)
            for j in range(3, NBLK):
                xbf_dma(j)

            def cast_x8(j, pair=None):
                # DVE scalar-mult cast bf16 -> fp8e4 with the SX pre-scale
                sl = slice(j * SBLK, (j + 1) * SBLK)
                if pair is None:
                    nc.vector.tensor_scalar_mul(
                        out=x8_sb[:, :, sl], in0=xbfs[j], scalar1=SX
                    )
                else:
                    nc.vector.tensor_scalar_mul(
                        out=x8_sb[:, 2 * pair : 2 * pair + 2, sl],
                        in0=xbfs[j][:, 2 * pair : 2 * pair + 2, :],
                        scalar1=SX,
                    )

            nc.vector.memset(cinv, INV)
            for p_ in range(NPAIR):
                nc.vector.memset(lhsT2_sb[p_], 0.0)
                nc.vector.memset(ksumrep_sb[p_], 0.0)

            # SBUF pools shared across both phases
            etpool = ctx.enter_context(tc.tile_pool(name="et", bufs=3))
            qtpool = ctx.enter_context(tc.tile_pool(name="qt", bufs=2))
            qts = {}

            def qt_elu(ps, j, f):
                # elu(z)+1 = min(exp(z),1) + relu(z), z = ps*INV (fp8
                # descale); Exp/Relu on ACT with the scale pre-op, the
                # combine on DVE
                e = etpool.tile([P, SBLK], F32, tag="e")
                nc.scalar.activation(out=e, in_=ps, func=ACT.Exp, scale=INV)
                r = etpool.tile([P, SBLK], F32, tag="r")
                nc.scalar.activation(out=r, in_=ps, func=ACT.Relu, scale=INV)
                nc.vector.scalar_tensor_tensor(
                    out=qts[j][:, f, :],
                    in0=e,
                    scalar=1.0,
                    in1=r,
                    op0=mybir.AluOpType.min,
                    op1=mybir.AluOpType.add,
                )

            def dr_mm(ps, lhsT, rhs, i):
                nc.tensor.matmul(
                    ps,
                    lhsT,
                    rhs,
                    start=(i == 0),
                    stop=(i == KPAIR - 1),
                    perf_mode=DR,
                )

            # ---------------- phase 1: K,V projection + KV accumulation ----
            with (
                tc.tile_pool(name="kvps", bufs=1, space="PSUM") as kvps_pool,
                tc.tile_pool(name="pa", bufs=4, space="PSUM") as pa_pool,
                tc.tile_pool(name="st", bufs=2) as stpool,
            ):
                kvps = [
                    kvps_pool.tile([P, P + 1], F32, tag=f"kv{p}", name=f"kv{p}")
                    for p in range(NPAIR)
                ]

                bq = []  # lagged [KV | K_sum] accumulation entries

                def emit_b(ent):
                    kst, vst, j, t = ent
                    first = j == 0 and t == 0
                    last = j == NBLK - 1 and t == TSUB - 1
                    for p_ in range(NPAIR):
                        nc.tensor.matmul(
                            kvps[p_],
                            kst[:, t, p_ * P : (p_ + 1) * P],
                            vst[:, t, p_, :],
                            start=first,
                            stop=last,
                        )

                def elu_k(ps, kst, t):
                    # e = exp(ps*INV) on ACT; r = max(ps,0)*INV on DVE via
                    # the cinv const tile; combine min(e,1)+r on DVE
                    e = etpool.tile([P, SBLK], F32, tag="e")
                    nc.scalar.activation(out=e, in_=ps, func=ACT.Exp, scale=INV)
                    r = etpool.tile([P, SBLK], F32, tag="r")
                    nc.vector.scalar_tensor_tensor(
                        out=r,
                        in0=ps,
                        scalar=0.0,
                        in1=cinv,
                        op0=mybir.AluOpType.max,
                        op1=mybir.AluOpType.mult,
                    )
                    nc.vector.scalar_tensor_tensor(
                        out=kst[:, t, :],
                        in0=e,
                        scalar=1.0,
                        in1=r,
                        op0=mybir.AluOpType.min,
                        op1=mybir.AluOpType.add,
                    )

                # block 0: K-sweep k-pair-outer so PE work tracks DMA chunk
                # arrival (4 simultaneous PSUM chains, one per 128-token
                # subtile); V follows t-outer once wv/xbf0 have landed
                kst0 = stpool.tile([P, TSUB, FSH], BF16, tag="kst")
                vst0 = stpool.tile([P, TSUB, NPAIR, P + 1], BF16, tag="vst")
                nc.vector.memset(vst0[:, :, :, P : P + 1], 1.0)
                psks = [
                    pa_pool.tile([P, SBLK], F32, tag="pa", name=f"psk{t}")
                    for t in range(TSUB)
                ]
                for i in range(KPAIR):
                    cast_x8(0, pair=i)
                    for t in range(TSUB):
                        dr_mm(
                            psks[t],
                            x8_sb[:, 2 * i : 2 * i + 2, t * P : (t + 1) * P],
                            wqk8_sb[:, 2 * i : 2 * i + 2, FSH : 2 * FSH],
                            i,
                        )
                for t in range(TSUB):
                    elu_k(psks[t], kst0, t)
                for t in range(TSUB):
                    psv = pa_pool.tile([P, SBLK], F32, tag="pa", name=f"psv{t}")
                    for k in range(KSUB):
                        nc.tensor.matmul(
                            psv,
                            xbf0[:, k, t * P : (t + 1) * P],
                            wv_sb[:, k, :],
                            start=(k == 0),
                            stop=(k == KSUB - 1),
                        )
                    if t == 0:
                        cast_x8(1)
                    if t == 2:
                        cast_x8(2)
                    if t >= 1:
                        emit_b(bq.pop(0))
                    nc.scalar.copy(out=vst0[:, t, :, 0:P], in_=psv)
                    bq.append((kst0, vst0, 0, t))

                # blocks 1..7: token-subtile-outer, B lagged one step
                for j in range(1, NBLK):
                    kst = stpool.tile([P, TSUB, FSH], BF16, tag="kst")
                    vst = stpool.tile([P, TSUB, NPAIR, P + 1], BF16, tag="vst")
                    nc.vector.memset(vst[:, :, :, P : P + 1], 1.0)
                    xbf = xbfs[j]
                    if j + 2 < NBLK:
                        # prefetch the cast two blocks ahead: the DVE FIFO
                        # runs about one block behind the PE in phase 1, so
                        # a one-block prefetch still lands on the
                        # K-projection critical path
                        cast_x8(j + 2)
                    for t in range(TSUB):
                        tok = j * SBLK + t * P
                        psk = pa_pool.tile([P, SBLK], F32, tag="pa")
                        psv = pa_pool.tile([P, SBLK], F32, tag="pa")
                        for i in range(KPAIR):
                            dr_mm(
                                psk,
                                x8_sb[:, 2 * i : 2 * i + 2, tok : tok + P],
                                wqk8_sb[:, 2 * i : 2 * i + 2, FSH : 2 * FSH],
                                i,
                            )
                        for k in range(KSUB):
                            nc.tensor.matmul(
                                psv,
                                xbf[:, k, t * P : (t + 1) * P],
                                wv_sb[:, k, :],
                                start=(k == 0),
                                stop=(k == KSUB - 1),
                            )
                        emit_b(bq.pop(0))
                        elu_k(psk, kst, t)
                        nc.scalar.copy(out=vst[:, t, :, 0:P], in_=psv)
                        bq.append((kst, vst, j, t))
                # block 0's Q projection runs here, inside the phase-1 PSUM
                # pools: it has no dependency on the KV state, so it keeps
                # the PE busy across the phase boundary
                qts[0] = qtpool.tile([P, NPAIR, SBLK], BF16, tag="qt", name="qt0")
                for f in range(FSH // P):
                    psq = pa_pool.tile([P, SBLK], F32, tag="pa")
                    for i in range(KPAIR):
                        dr_mm(
                            psq,
                            wqk8_sb[:, 2 * i : 2 * i + 2, f * P : (f + 1) * P],
                            x8_sb[:, 2 * i : 2 * i + 2, 0:SBLK],
                            i,
                        )
                    if f == 0:
                        emit_b(bq.pop(0))
                    # extraction spread one pair per f iteration: the DVE
                    # copies hide under the Q-chain matmuls without ever
                    # queuing ahead of the elu STTs
                    p_ = f
                    nc.vector.tensor_copy(
                        out=lhsT2_sb[p_][0:HD, 0:HD], in_=kvps[p_][0:HD, 0:HD]
                    )
                    nc.vector.tensor_copy(
                        out=lhsT2_sb[p_][HD:P, HD:P], in_=kvps[p_][HD:P, HD:P]
                    )
                    nc.vector.tensor_copy(
                        out=ksumrep_sb[p_][0:HD, 0:HD],
                        in_=kvps[p_][0:HD, P : P + 1].to_broadcast((HD, HD)),
                    )
                    nc.vector.tensor_copy(
                        out=ksumrep_sb[p_][HD:P, HD:P],
                        in_=kvps[p_][HD:P, P : P + 1].to_broadcast((HD, HD)),
                    )
                    qt_elu(psq, 0, f)

            # ---------------- phase 2: Q projection + attention + Wo -------
            with (
                tc.tile_pool(name="mm512", bufs=4, space="PSUM") as mmps,
                tc.tile_pool(name="pc", bufs=2, space="PSUM") as pcps,
                tc.tile_pool(name="pnb", bufs=2, space="PSUM") as pnps,
                tc.tile_pool(name="ou", bufs=3) as oupool,
                tc.tile_pool(name="rc", bufs=4) as rcpool,
                tc.tile_pool(name="ot", bufs=2) as otpool,
                tc.tile_pool(name="ys", bufs=4) as ypool,
            ):
                outus = {}
                rcbs = {}
                outts = {}

                def qt_half(j, fh):
                    if j not in qts:
                        qts[j] = qtpool.tile(
                            [P, NPAIR, SBLK], BF16, tag="qt", name=f"qt{j}"
                        )
                    for f in (2 * fh, 2 * fh + 1):
                        ps = mmps.tile([P, SBLK], F32, tag="mm")
                        for i in range(KPAIR):
                            dr_mm(
                                ps,
                                wqk8_sb[:, 2 * i : 2 * i + 2, f * P : (f + 1) * P],
                                x8_sb[:, 2 * i : 2 * i + 2, j * SBLK : (j + 1) * SBLK],
                                i,
                            )
                        qt_elu(ps, j, f)

                def attn_pairs(j, pairs):
                    # per pair: attention matmul + broadcast-normalizer
                    # matmul; the DVE apply-multiply reads psc directly from
                    # PSUM (no ACT eviction, one less latency link)
                    qtj = qts[j]
                    if j not in outts:
                        outts[j] = otpool.tile(
                            [P, NPAIR, SBLK], BF16, tag="outt", name="outt"
                        )
                    outt = outts[j]
                    for p_ in pairs:
                        psc = pcps.tile([P, SBLK], F32, tag="pc", name="psc")
                        nc.tensor.matmul(
                            psc,
                            lhsT2_sb[p_],
                            qtj[:, p_, :],
                            start=True,
                            stop=True,
                        )
                        psn = pnps.tile([P, SBLK], F32, tag="pn", name="psn")
                        nc.tensor.matmul(
                            psn,
                            ksumrep_sb[p_],
                            qtj[:, p_, :],
                            start=True,
                            stop=True,
                        )
                        rcb = rcpool.tile([P, SBLK], F32, tag="rcb", name="rcb")
                        nc.vector.reciprocal_approx_fast(out=rcb[:], in_=psn[:])
                        nc.vector.tensor_tensor(
                            out=outt[:, p_, :],
                            in0=psc[:],
                            in1=rcb[:],
                            op=mybir.AluOpType.mult,
                        )

                def psc_section(j):
                    # finale-only variant: ACT-evicts psc to outu so the
                    # apply can be split per token subtile in the drain
                    qtj = qts.pop(j)
                    outu = oupool.tile([P, NPAIR, SBLK], F32, tag="outu")
                    outus[j] = outu
                    rcbs[j] = []
                    for p_ in range(NPAIR):
                        psc = pcps.tile([P, SBLK], F32, tag="pc")
                        nc.tensor.matmul(
                            psc,
                            lhsT2_sb[p_],
                            qtj[:, p_, :],
                            start=True,
                            stop=True,
                        )
                        nc.scalar.copy(out=outu[:, p_, :], in_=psc)
                        psn = pnps.tile([P, SBLK], F32, tag="pn")
                        nc.tensor.matmul(
                            psn,
                            ksumrep_sb[p_],
                            qtj[:, p_, :],
                            start=True,
                            stop=True,
                        )
                        rcb = rcpool.tile([P, SBLK], F32, tag="rcb")
                        nc.vector.reciprocal_approx_fast(out=rcb[:], in_=psn[:])
                        rcbs[j].append(rcb)

                def d_t(j, outt, t, drain=False):
                    ysb = ypool.tile([P, D], F32, tag="ysb", name="ysb")
                    psy0 = mmps.tile([P, 512], F32, tag="mm", name="psy0")
                    psy1 = mmps.tile([P, 512], F32, tag="mm", name="psy1")
                    for ps_, off in ((psy0, 0), (psy1, 512)) if drain else (
                        (None, None),
                    ):
                        if ps_ is None:
                            break
                        for fs in range(FSH // P):
                            nc.tensor.matmul(
                                ps_,
                                outt[:, fs, t * P : (t + 1) * P],
                                wo_sb[:, fs, off : off + 512],
                                start=(fs == 0),
                                stop=(fs == FSH // P - 1),
                            )
                    if not drain:
                        for fs in range(FSH // P):
                            nc.tensor.matmul(
                                psy0,
                                outt[:, fs, t * P : (t + 1) * P],
                                wo_sb[:, fs, 0:512],
                                start=(fs == 0),
                                stop=(fs == FSH // P - 1),
                            )
                            nc.tensor.matmul(
                                psy1,
                                outt[:, fs, t * P : (t + 1) * P],
                                wo_sb[:, fs, 512:1024],
                                start=(fs == 0),
                                stop=(fs == FSH // P - 1),
                            )
                    # both evictions on ACT: DVE is the congested FIFO in
                    # phase 2 (elu STT + reciprocal + apply), so PSUM-slot
                    # release must not ride it
                    nc.scalar.copy(out=ysb[:, 0:512], in_=psy0)
                    if drain:
                        nc.sync.dma_start(out=y_rt[j, t, 0], in_=ysb[:, 0:512])
                        nc.vector.tensor_copy(out=ysb[:, 512:1024], in_=psy1)
                        nc.sync.dma_start(out=y_rt[j, t, 1], in_=ysb[:, 512:1024])
                    else:
                        nc.scalar.copy(out=ysb[:, 512:1024], in_=psy1)
                        nc.sync.dma_start(out=y_rb[j, t], in_=ysb)

                def d_half(j, ts):
                    for t in ts:
                        d_t(j, outts[j], t)

                def finale(j):
                    # drain block: apply-multiplies split per token subtile
                    # so each D chain starts as soon as its slice is scaled
                    outt = otpool.tile([P, NPAIR, SBLK], BF16, tag="outt")
                    outu = outus.pop(j)
                    rcs = rcbs.pop(j)
                    for t in range(TSUB):
                        sl = slice(t * P, (t + 1) * P)
                        for p_ in range(NPAIR):
                            nc.vector.tensor_tensor(
                                out=outt[:, p_, sl],
                                in0=outu[:, p_, sl],
                                in1=rcs[p_][:, sl],
                                op=mybir.AluOpType.mult,
                            )
                        d_t(j, outt, t, drain=(t == TSUB - 1))

                # steady-state emission: block j's Q projection brackets
                # block j-1's attention chain so the PE never waits on the
                # ACT/DVE eviction+reciprocal+apply latency; the out-proj
                # subtiles interleave between the Q halves so block
                # boundaries never wait on the last qt elu.
                for j in range(1, NBLK):
                    attn_pairs(j - 1, [0, 1])
                    attn_pairs(j - 1, [2, 3])
                    qt_half(j, 0)
                    d_half(j - 1, [0, 1])
                    qt_half(j, 1)
                    if j == NBLK - 1:
                        # last block's attention section runs ahead of the
                        # final d_t's so its evictions and reciprocals hide
                        # under D's matmuls and the drain starts immediately
                        psc_section(NBLK - 1)
                    d_half(j - 1, [2, 3])
                    outts.pop(j - 1)
                finale(NBLK - 1)

    nc.compile()
    return nc


def _prep_inputs(x, Wqkv, Wo):
    import ml_dtypes

    x = np.ascontiguousarray(x, dtype=np.float32)
    Wqkv = np.ascontiguousarray(Wqkv, dtype=np.float32)
    Wo = np.ascontiguousarray(Wo, dtype=np.float32)

    def f8(a):
        return np.clip(a, -240.0, 240.0).astype(ml_dtypes.float8_e4m3fn)

    in_maps = []
    for b in range(B):
        xT = np.ascontiguousarray(x[b].T).astype(ml_dtypes.bfloat16)  # [D, S]
        for hh in range(2):
            cols = slice(hh * FSH, (hh + 1) * FSH)
            wq = Wqkv[:, 0 * D :][:, cols]
            wk = Wqkv[:, 1 * D :][:, cols]
            wv = Wqkv[:, 2 * D :][:, cols]
            wqk8 = f8(
                np.ascontiguousarray(np.concatenate([wq, wk], axis=1)) * SW
            )
            wv_sh = np.ascontiguousarray(wv).astype(ml_dtypes.bfloat16)
            wo_sh = np.ascontiguousarray(Wo[hh * FSH : (hh + 1) * FSH, :]).astype(
                ml_dtypes.bfloat16
            )
            in_maps.append(
                {"xT": xT, "wqk8": wqk8, "wv": wv_sh, "wo": wo_sh}
            )
    return in_maps


def kernel(x, Wqkv, Wo):
    global _NC_CACHE
    if _NC_CACHE is None:
        _NC_CACHE = build()
    nc = _NC_CACHE
    in_maps = _prep_inputs(x, Wqkv, Wo)
    res = run_bass_kernel_spmd(nc, in_maps, list(range(2 * B))).results
    y = np.empty((B, S, D), dtype=np.float32)
    for b in range(B):
        y[b] = res[2 * b]["y"] + res[2 * b + 1]["y"]
    return y
